# revision 20
# baseline (speedup 1.0000x reference)
"""Trainium2 Bass kernel for the AttractorNetwork LIF recurrent scan.

Strategy (8 NeuronCores, one chip):
  - Model-parallel over output neurons: each core owns a 256-neuron j-shard
    and the full batch (128). Weights live in SBUF as bf16 stationary tiles.
  - Per timestep each core computes rec^T[j_local, b] = sum_i W[i, j] spk^T[i, b]
    with 32 (LDWEIGHTS+MATMUL N=128) pairs accumulating in PSUM, then the LIF
    membrane update on VectorE, then broadcasts its binary spike tile (bf16,
    64 KB) to all peers with remote SBUF-to-SBUF DMA (XOR slot trick so the
    SPMD program is identical on every core: sender s writes peer d = s^k at
    static slot k; the host permutes each core's weight blocks to match).
  - The cue is folded into the noise on the host (noise'[t<cue_d] += cue), so
    the device loop is: mem = mem*decay + rec + noise; spk = mem >= 1;
    mem *= (mem < 1); acc += spk (second half only).
"""

import sys

sys.path.insert(0, "/opt/trn_rl_repo")

import numpy as np
import ml_dtypes

import concourse.bass as bass
import concourse.mybir as mybir
from concourse import library_config
from concourse.bacc import Bacc
from concourse.bass_utils import run_bass_kernel_spmd

F32 = mybir.dt.float32
BF16 = mybir.dt.bfloat16
OP = mybir.AluOpType

N = 2048
B = 128
NCORES = 8
J = N // NCORES          # 256 neurons per core
TAU_MEM = 20.0
DT_ = 1.0
INHIBITION = 0.1
V_THRESH = 1.0
CUE_STRENGTH = 1.0
DECAY = float(np.float32(np.exp(-DT_ / TAU_MEM)))
CHUNK = 4                # noise steps per DMA
RING = 4                 # chunks resident in the SBUF noise ring


def build_nc(T, debug=False, lowering=True, drains=True):
    """Build the SPMD Bass program for T timesteps."""
    half = T // 2
    nchunks = (T + CHUNK - 1) // CHUNK

    if lowering:
        nc = Bacc(debug=debug)
    else:
        nc = bass.Bass(debug=debug, target_bir_lowering=False)

    wq = nc.declare_dram_parameter("wq", [128, 16 * J], BF16, isOutput=False)
    noise_d = nc.declare_dram_parameter(
        "noise", [nchunks, 128, CHUNK * J], F32, isOutput=False)
    out_d = nc.declare_dram_parameter("out", [128, J], F32, isOutput=True)

    from contextlib import ExitStack
    with ExitStack() as es:
        w_sb = es.enter_context(nc.sbuf_tensor("w_sb", [128, 16 * J], BF16))
        ring = es.enter_context(
            nc.sbuf_tensor("ring", [128, RING * CHUNK * J], F32))
        spk0 = es.enter_context(nc.sbuf_tensor("spk0", [128, N], BF16))
        spk1 = es.enter_context(nc.sbuf_tensor("spk1", [128, N], BF16))
        mem = es.enter_context(nc.sbuf_tensor("mem", [128, J], F32))
        u = es.enter_context(nc.sbuf_tensor("u", [128, J], F32))
        acc = es.enter_context(nc.sbuf_tensor("acc", [128, J], F32))
        ps00 = es.enter_context(nc.psum_tensor("ps00", [128, 128], F32))
        ps01 = es.enter_context(nc.psum_tensor("ps01", [128, 128], F32))
        ps10 = es.enter_context(nc.psum_tensor("ps10", [128, 128], F32))
        ps11 = es.enter_context(nc.psum_tensor("ps11", [128, 128], F32))
        w_sem = es.enter_context(nc.semaphore("w_sem"))
        noise_rdy = [
            es.enter_context(nc.semaphore(f"noise_rdy{i}")) for i in range(RING)
        ]
        noise_cons = es.enter_context(nc.semaphore("noise_cons"))
        mm_done = es.enter_context(nc.semaphore("mm_done"))
        psum_free = es.enter_context(nc.semaphore("psum_free"))
        spk_own = es.enter_context(nc.semaphore("spk_own"))
        recv_sem = [
            es.enter_context(nc.semaphore(f"recv{k}")) for k in range(1, NCORES)
        ]
        send_local = es.enter_context(nc.semaphore("send_local"))
        prep_sem = es.enter_context(nc.semaphore("prep_sem"))
        odma = es.enter_context(nc.semaphore("odma"))
        block = es.enter_context(nc.Block())
        spk_buf = [spk0, spk1]
        ps = [[ps00, ps01], [ps10, ps11]]

        def noise_ap(t, h):
            # step t's noise for half h: ring col (t % (RING*CHUNK))*J + h*128
            c = (t % (RING * CHUNK)) * J + h * 128
            return ring[:, c:c + 128]

        def wblk(k, hi, h):
            i = ((k * 2 + hi) * 2 + h) * 128
            return w_sb[:, i:i + 128]

        @block.sync
        def _(sync):
            sync.dma_start(out=w_sb[:, :], in_=wq[:, :]).then_inc(w_sem, 16)
            for c in range(nchunks):
                if c >= RING:
                    # slot reuse: steps of chunk c-RING consumed through
                    # step (c-RING)*CHUNK + CHUNK - 1
                    sync.wait_ge(noise_cons, (c - RING) * CHUNK + CHUNK)
                s = (c % RING) * CHUNK * J
                sync.dma_start(
                    out=ring[:, s:s + CHUNK * J], in_=noise_d[c]
                ).then_inc(noise_rdy[c % RING], 16)
            sync.wait_ge(noise_cons, T)
            sync.dma_start(out=out_d[:, :], in_=acc[:, :]).then_inc(odma, 16)
            sync.wait_ge(odma, 16)

        @block.tensor
        def _(tensor):
            tensor.wait_ge(w_sem, 16)
            for t in range(1, T):
                par = t % 2
                ppar = (t - 1) % 2
                if t >= 3:
                    tensor.wait_ge(psum_free, t - 2)
                for k in range(NCORES):
                    if k == 0:
                        tensor.wait_ge(spk_own, t)
                    else:
                        tensor.wait_ge(recv_sem[k - 1], 2 * t)
                    for hi in range(2):
                        for h in range(2):
                            mm = tensor.matmul(
                                ps[par][h][:, :],
                                wblk(k, hi, h),
                                spk_buf[ppar][:, J * k + 128 * hi:
                                              J * k + 128 * hi + 128],
                                start=(k == 0 and hi == 0),
                                stop=(k == NCORES - 1 and hi == 1),
                            )
                            if k == NCORES - 1 and hi == 1:
                                mm.then_inc(mm_done, 1)

        @block.vector
        def _(vector):
            vector.memset(acc[:, :], 0.0)
            # t = 0: mem was 0, rec = 0 -> membrane is just noise'(0)
            vector.wait_ge(noise_rdy[0], 16)
            for h in range(2):
                na = noise_ap(0, h)
                ts = vector.tensor_scalar(
                    spk_buf[0][:, 128 * h:128 * h + 128], na,
                    V_THRESH, None, OP.is_ge)
                if h == 1:
                    ts.then_inc(spk_own, 1)
                st = vector.scalar_tensor_tensor(
                    mem[:, 128 * h:128 * h + 128], na, V_THRESH, na,
                    OP.is_lt, OP.mult)
            st.then_inc(noise_cons, 1)

            for t in range(1, T):
                par = t % 2
                if t % CHUNK == 0:
                    c = t // CHUNK
                    vector.wait_ge(noise_rdy[c % RING], 16 * (c // RING + 1))
                if drains:
                    vector.drain()   # order: mem/acc writes of step t-1
                # leak + noise for both halves first (independent of matmul)
                for h in range(2):
                    hs = slice(128 * h, 128 * h + 128)
                    vector.scalar_tensor_tensor(
                        u[:, hs], mem[:, hs], DECAY, noise_ap(t, h),
                        OP.mult, OP.add)
                if t >= 2 and t < T - 1:
                    # sends of step t-2 must have finished reading slot 0
                    # of parity t%2 before we overwrite it
                    vector.wait_ge(send_local, 112 * (t - 1))
                if drains:
                    vector.drain()   # order: u writes by the leak ops
                for h in range(2):
                    hs = slice(128 * h, 128 * h + 128)
                    vector.wait_ge(mm_done, 2 * (t - 1) + h + 1)
                    tt = vector.tensor_tensor(
                        u[:, hs], u[:, hs], ps[par][h][:, :], OP.add)
                    if h == 1:
                        tt.then_inc(psum_free, 1)
                    if drains:
                        vector.drain()   # order: u halves updated with rec
                    if t < T - 1:
                        ts = vector.tensor_scalar(
                            spk_buf[par][:, hs], u[:, hs],
                            V_THRESH, None, OP.is_ge)
                        if h == 1:
                            ts.then_inc(spk_own, 1)
                    if t >= half:
                        vector.scalar_tensor_tensor(
                            acc[:, hs], u[:, hs], V_THRESH, acc[:, hs],
                            OP.is_ge, OP.add)
                    st = vector.scalar_tensor_tensor(
                        mem[:, hs], u[:, hs], V_THRESH, u[:, hs],
                        OP.is_lt, OP.mult)
                st.then_inc(noise_cons, 1)

        @block.gpsimd
        def _(gpsimd):
            if not lowering:
                gpsimd.load_library(library_config.remote_dma)
            for t in range(T - 1):
                par = t % 2
                gpsimd.wait_ge(spk_own, t + 1)
                for k in range(1, NCORES):
                    rd = [None] * 8
                    rd[k] = (0, k)
                    gpsimd.remote_dma_broadcast(
                        out_ap=spk_buf[par][:, J * k:J * k + J],
                        in_ap=spk_buf[par][:, 0:J],
                        remote_sem=recv_sem[k - 1],
                        local_sem=send_local,
                        rdests=rd,
                    ).then_inc(prep_sem, 1)
                gpsimd.wait_ge(prep_sem, 7 * (t + 1))
                gpsimd.trigger_dma(count=7)

    return nc


# Logical jax-device index -> physical TPB index on trn2 (die-1 SEngines are
# swapped in the physical numbering; measured with probe_test.py). The rdests
# XOR routing operates on PHYSICAL tpb ids, so logical core r is assigned the
# data of "algorithm core" PHYS[r]; in algorithm-id space the slot relation is
# then exactly slot k <-> peer (q ^ k).
PHYS = [0, 1, 2, 3, 6, 7, 4, 5]


def prep_inputs(cue, weights, noise, T, cue_duration, phys=None):
    """Host-side sharding: returns in_maps for run_bass_kernel_spmd."""
    cue = np.asarray(cue, np.float32)
    weights = np.asarray(weights, np.float32)
    noise = np.asarray(noise, np.float32)

    w_eff = (weights - np.float32(INHIBITION / N)) * (
        1.0 - np.eye(N, dtype=np.float32))

    noise_eff = noise.copy()
    noise_eff[:cue_duration] += np.float32(CUE_STRENGTH) * cue

    nchunks = (T + CHUNK - 1) // CHUNK
    pad = nchunks * CHUNK - T
    if pad:
        noise_eff = np.concatenate(
            [noise_eff, np.zeros((pad, B, N), np.float32)], axis=0)

    if phys is None:
        phys = list(range(NCORES))
    in_maps = []
    for lr in range(NCORES):
        r = phys[lr]
        jsl = slice(J * r, J * r + J)
        # weight blocks, slot-permuted: slot k holds rows of sender s = r^k
        blocks = np.empty((128, 16 * J), np.float32)
        for k in range(NCORES):
            s = r ^ k
            blk = w_eff[J * s:J * s + J, jsl]          # [256 i, 256 j]
            # [hi, p, h, jj] -> [p, (hi, h, jj)]
            b4 = blk.reshape(2, 128, 2, 128).transpose(1, 0, 2, 3)
            blocks[:, k * 4 * 128:(k + 1) * 4 * 128] = b4.reshape(128, 512)
        wq = blocks.astype(ml_dtypes.bfloat16)

        # noise: [t, b, jglobal] -> [chunk, p, (q, h, b)]
        nz = noise_eff[:, :, jsl]                      # [Tp, 128b, 256j]
        nz = nz.transpose(0, 2, 1)                     # [Tp, 256j, 128b]
        nz = nz.reshape(nchunks, CHUNK, 2, 128, B)     # [c, q, h, p, b]
        nz = nz.transpose(0, 3, 1, 2, 4)               # [c, p, q, h, b]
        nz = np.ascontiguousarray(nz.reshape(nchunks, 128, CHUNK * J),
                                  dtype=np.float32)
        in_maps.append({"wq": wq, "noise": nz})
    return in_maps


def assemble_output(outs, T, phys=None):
    """outs: list of per-core {"out": [128, 256]} -> [B, N] mean activity."""
    if phys is None:
        phys = list(range(NCORES))
    half = T // 2
    mean = np.empty((B, N), np.float32)
    for lr in range(NCORES):
        r = phys[lr]
        oc = np.asarray(outs[lr]["out"], np.float32)   # [p, h*128 + b]
        oc = oc.reshape(128, 2, B).transpose(1, 0, 2)  # [h, p, b]
        mean[:, J * r:J * r + J] = oc.reshape(J, B).T
    return mean / np.float32(half)


_NC_CACHE = {}


def _ensure_ntff_hook():
    """The agent image's antenv lacks axon_hooks; recreate it so
    run_bass_kernel_spmd(trace=True) can capture NTFF profiles."""
    import types
    import ctypes
    import contextlib
    try:
        from antenv.axon_hooks import get_axon_ntff_profile_hook  # noqa: F401
        return
    except ImportError:
        pass
    so_path = "/opt/axon/libaxon_pjrt.so"
    try:
        lib = ctypes.CDLL(so_path)
        if not hasattr(lib, "axon_start_nrt_profile"):
            return
    except OSError:
        return
    lib.axon_start_nrt_profile.argtypes = [
        ctypes.POINTER(ctypes.c_int64), ctypes.c_size_t]
    lib.axon_start_nrt_profile.restype = ctypes.c_int64
    lib.axon_stop_nrt_profile.argtypes = [ctypes.c_char_p]
    lib.axon_stop_nrt_profile.restype = ctypes.c_int64

    @contextlib.contextmanager
    def _hook(output_dir, device_ids):
        import jax
        jax.devices()
        if device_ids:
            ids = (ctypes.c_int64 * len(device_ids))(*device_ids)
            rc = lib.axon_start_nrt_profile(ids, len(device_ids))
        else:
            rc = lib.axon_start_nrt_profile(None, 0)
        if rc != 0:
            raise RuntimeError(f"axon_start_nrt_profile rc={rc}")
        try:
            yield
        finally:
            n = lib.axon_stop_nrt_profile(str(output_dir).encode())
            if n < 0:
                raise RuntimeError(f"axon_stop_nrt_profile rc={n}")

    mod = types.ModuleType("antenv.axon_hooks")
    mod._hook = _hook
    mod.get_axon_ntff_profile_hook = lambda: mod._hook
    mod.set_axon_ntff_profile_hook = lambda h: setattr(mod, "_hook", h)
    sys.modules["antenv.axon_hooks"] = mod


def kernel(cue, weights, noise, steps, cue_duration, trace=False):
    T = int(steps)
    cd = int(cue_duration)
    in_maps = prep_inputs(cue, weights, noise, T, cd, phys=PHYS)
    if T not in _NC_CACHE:
        nc_new = build_nc(T)
        nc_new.finalize()
        _NC_CACHE[T] = nc_new
    nc = _NC_CACHE[T]
    if trace:
        _ensure_ntff_hook()
    res = run_bass_kernel_spmd(nc, in_maps, list(range(NCORES)), trace=trace)
    out = assemble_output(res.results, T, phys=PHYS)
    kernel.last_result = res
    return out


# revision 33
# speedup vs baseline: 20.8166x; 20.8166x over previous
"""Trainium2 Bass kernel for the AttractorNetwork LIF recurrent scan.

Strategy (8 NeuronCores, one chip): pure data-parallel over batch, ZERO
cross-core communication. Each core owns 16 batch rows and keeps the full
[2048, 2048] effective weight matrix in SBUF as bf16 (8 MB). Per timestep:

  rec[b, :] = spk[b, :] @ w_eff        -> 64 matmuls (N=512, M=16), packed
                                          4-wide into PE column groups via
                                          tile_position so four j-slices
                                          compute concurrently
  rec^T chunks via 16 PE-transposes    -> back into the [neuron, batch]
                                          state layout (128 partitions)
  LIF update on VectorE (full width)   -> mem, spike, reset, activity accum

The cue is folded into the noise on the host (noise'[t<cue_d] += cue), so
the device loop is: u = mem*decay + rec + noise; spk = u >= 1;
mem = u * (u < 1); acc += spk (second half only). Spikes are stored as bf16
{0,1} (exact) and feed the next step's matmul as the stationary operand;
weights are bf16 (validated: output identical for the task's inputs).

Cores never exchange data, so there is no sensitivity to launch skew or
cross-core latency; the harness gathers per-core [128, 256] activity
accumulators and reassembles the [128, 2048] mean-activity output.
"""

import sys

sys.path.insert(0, "/opt/trn_rl_repo")

import numpy as np
import ml_dtypes

import concourse.bass as bass
import concourse.mybir as mybir
from concourse.bacc import Bacc
from concourse.bass_utils import run_bass_kernel_spmd

F32 = mybir.dt.float32
BF16 = mybir.dt.bfloat16
OP = mybir.AluOpType

N = 2048
B = 128
NCORES = 8
BL = B // NCORES         # 16 batch rows per core
NT = N // 128            # 16 neuron tiles
TAU_MEM = 20.0
DT_ = 1.0
INHIBITION = 0.1
V_THRESH = 1.0
CUE_STRENGTH = 1.0
DECAY = float(np.float32(np.exp(-DT_ / TAU_MEM)))
CHUNK = 4                # noise steps per DMA
RING = 4                 # chunks resident in the SBUF noise ring
F = NT * BL              # 256: state free width ([p, jt*16+b])


def build_nc(T, debug=False, lowering=True, drains=True):
    """Build the (SPMD but communication-free) Bass program for T steps."""
    half = T // 2
    nchunks = (T + CHUNK - 1) // CHUNK

    if lowering:
        nc = Bacc(debug=debug)
    else:
        nc = bass.Bass(debug=debug, target_bir_lowering=False)

    wq = nc.declare_dram_parameter("wq", [128, NT * N], BF16, isOutput=False)
    noise_d = nc.declare_dram_parameter(
        "noise", [nchunks, 128, CHUNK * F], F32, isOutput=False)
    ident_d = nc.declare_dram_parameter("ident_d", [128, 64], F32, isOutput=False)
    out_d = nc.declare_dram_parameter("out", [128, F], F32, isOutput=True)

    from contextlib import ExitStack
    with ExitStack() as es:
        w_sb = es.enter_context(nc.sbuf_tensor("w_sb", [128, NT * N], BF16))
        ring = es.enter_context(
            nc.sbuf_tensor("ring", [128, RING * CHUNK * F], F32))
        rec_sb = es.enter_context(nc.sbuf_tensor("rec_sb", [128, 512], F32))
        ident = es.enter_context(nc.sbuf_tensor("ident", [128, 64], F32))
        spk0 = es.enter_context(nc.sbuf_tensor("spk0", [128, F], BF16))
        spk1 = es.enter_context(nc.sbuf_tensor("spk1", [128, F], BF16))
        mem = es.enter_context(nc.sbuf_tensor("mem", [128, F], F32))
        u = es.enter_context(nc.sbuf_tensor("u", [128, F], F32))
        acc = es.enter_context(nc.sbuf_tensor("acc", [128, F], F32))
        psr0 = es.enter_context(nc.psum_tensor("psr0", [128, 512], F32))
        psr1 = es.enter_context(nc.psum_tensor("psr1", [128, 512], F32))
        pst0 = es.enter_context(nc.psum_tensor("pst0", [128, 512], F32))
        pst1 = es.enter_context(nc.psum_tensor("pst1", [128, 512], F32))
        w_sem = es.enter_context(nc.semaphore("w_sem"))
        noise_rdy = [
            es.enter_context(nc.semaphore(f"noise_rdy{i}")) for i in range(RING)
        ]
        noise_cons = es.enter_context(nc.semaphore("noise_cons"))
        mm_done = es.enter_context(nc.semaphore("mm_done"))
        copy_sem = es.enter_context(nc.semaphore("copy_sem"))
        tp_done = es.enter_context(nc.semaphore("tp_done"))
        spk_own = es.enter_context(nc.semaphore("spk_own"))
        odma = es.enter_context(nc.semaphore("odma"))
        block = es.enter_context(nc.Block())

        spk_buf = [spk0, spk1]
        ps_rec = [psr0, psr1]
        ps_spk = [pst0, pst1]

        def noise_ap(t):
            c = (t % (RING * CHUNK)) * F
            return ring[:, c:c + F]

        @block.sync
        def _(sync):
            sync.dma_start(out=ident[:, :], in_=ident_d[:, :]).then_inc(w_sem, 16)
            sync.dma_start(out=w_sb[:, :], in_=wq[:, :]).then_inc(w_sem, 16)
            for c in range(nchunks):
                if c >= RING:
                    sync.wait_ge(noise_cons, (c - RING) * CHUNK + CHUNK)
                s = (c % RING) * CHUNK * F
                sync.dma_start(
                    out=ring[:, s:s + CHUNK * F], in_=noise_d[c]
                ).then_inc(noise_rdy[c % RING], 16)
            sync.wait_ge(noise_cons, T)
            sync.dma_start(out=out_d[:, :], in_=acc[:, :]).then_inc(odma, 16)
            sync.wait_ge(odma, 16)

        @block.tensor
        def _(tensor):
            tensor.wait_ge(w_sem, 32)
            for t in range(1, T):
                par = t % 2
                ppar = (t - 1) % 2
                tensor.wait_ge(spk_own, t)
                for i in range(NT):
                    for g in range(4):
                        mm = tensor.matmul(
                            ps_rec[par][32 * g:32 * g + BL, :],
                            spk_buf[ppar][:, BL * i:BL * i + BL],
                            w_sb[:, N * i + 512 * g:N * i + 512 * g + 512],
                            start=(i == 0),
                            stop=(i == NT - 1),
                            tile_position=(0, 32 * g),
                            skip_group_check=True,
                        )
                mm.then_inc(mm_done, 1)
                tensor.wait_ge(copy_sem, t)
                # transpose the four 16-row rec bands back to the state
                # layout with one exact 0/1 selection matmul per 128-col
                # chunk: psum col 64c+16g+b = rec_sb[32g+b, 128c+x]
                for cc in range(4):
                    tp = tensor.matmul(
                        ps_spk[par][:, 64 * cc:64 * cc + 64],
                        rec_sb[:, 128 * cc:128 * cc + 128],
                        ident[:, :],
                        start=True, stop=True,
                        skip_group_check=True,
                    )
                tp.then_inc(tp_done, 1)

        @block.vector
        def _(vector):
            vector.memset(acc[:, :], 0.0)
            # zero the never-written partition bands of the matmul PSUM so
            # the full-width rec copy reads defined data
            vector.memset(ps_rec[0][:, :], 0.0)
            vector.memset(ps_rec[1][:, :], 0.0)
            # t = 0: mem was 0, rec = 0 -> membrane is just noise'(0)
            vector.wait_ge(noise_rdy[0], 16)
            na = noise_ap(0)
            vector.tensor_scalar(
                spk_buf[0][:, :], na, V_THRESH, None, OP.is_ge
            ).then_inc(spk_own, 1)
            vector.scalar_tensor_tensor(
                mem[:, :], na, V_THRESH, na, OP.is_lt, OP.mult
            ).then_inc(noise_cons, 1)

            for t in range(1, T):
                par = t % 2
                if t % CHUNK == 0:
                    c = t // CHUNK
                    vector.wait_ge(noise_rdy[c % RING], 16 * (c // RING + 1))
                vector.wait_ge(mm_done, t)
                vector.tensor_copy(
                    rec_sb[:, :], ps_rec[par][:, :]).then_inc(copy_sem, 1)
                if drains:
                    vector.drain()   # order: mem/acc writes of step t-1
                vector.scalar_tensor_tensor(
                    u[:, :], mem[:, :], DECAY, noise_ap(t), OP.mult, OP.add)
                vector.wait_ge(tp_done, t)
                if drains:
                    vector.drain()   # order: u leak write
                # u free order is (g, c, b); psum cols are (c, g, b)
                u3 = u[:, :].rearrange("p (g c b) -> p g c b", g=4, c=4, b=BL)
                ps3 = ps_spk[par][:, 0:F].rearrange(
                    "p (c g b) -> p g c b", c=4, g=4, b=BL)
                vector.tensor_tensor(u3, u3, ps3, OP.add)
                if drains:
                    vector.drain()   # order: u += rec write
                if t < T - 1:
                    vector.tensor_scalar(
                        spk_buf[par][:, :], u[:, :], V_THRESH, None, OP.is_ge
                    ).then_inc(spk_own, 1)
                if t >= half:
                    vector.scalar_tensor_tensor(
                        acc[:, :], u[:, :], V_THRESH, acc[:, :],
                        OP.is_ge, OP.add)
                vector.scalar_tensor_tensor(
                    mem[:, :], u[:, :], V_THRESH, u[:, :], OP.is_lt, OP.mult
                ).then_inc(noise_cons, 1)

    return nc


def prep_inputs(cue, weights, noise, T, cue_duration):
    """Host-side sharding: returns in_maps for run_bass_kernel_spmd."""
    cue = np.asarray(cue, np.float32)
    weights = np.asarray(weights, np.float32)
    noise = np.asarray(noise, np.float32)

    w_eff = (weights - np.float32(INHIBITION / N)) * (
        1.0 - np.eye(N, dtype=np.float32))

    noise_eff = noise.copy()
    noise_eff[:cue_duration] += np.float32(CUE_STRENGTH) * cue

    nchunks = (T + CHUNK - 1) // CHUNK
    pad = nchunks * CHUNK - T
    if pad:
        noise_eff = np.concatenate(
            [noise_eff, np.zeros((pad, B, N), np.float32)], axis=0)

    # replicated weights: wq[p, i_tile*N + j] = w_eff[128*i_tile + p, j]
    wq = np.ascontiguousarray(
        w_eff.reshape(NT, 128, N).transpose(1, 0, 2).reshape(128, NT * N)
    ).astype(ml_dtypes.bfloat16)

    # 0/1 selection matrix for the rec-band transpose matmuls:
    # column 16g+b picks row 32g+b
    ident = np.zeros((128, 64), np.float32)
    for g in range(4):
        for b in range(BL):
            ident[32 * g + b, 16 * g + b] = 1.0

    in_maps = []
    for r in range(NCORES):
        bsl = slice(BL * r, BL * r + BL)
        # noise: [t, b, j] -> [chunk, p, (q, jt, b)]
        nz = noise_eff[:, bsl, :]                     # [Tp, 16, 2048]
        nz = nz.transpose(0, 2, 1)                    # [Tp, 2048j, 16b]
        nz = nz.reshape(-1, NT, 128, BL)              # [Tp, jt, p, b]
        nz = nz.transpose(0, 2, 1, 3)                 # [Tp, p, jt, b]
        nz = nz.reshape(nchunks, CHUNK, 128, F)       # [c, q, p, f]
        nz = nz.transpose(0, 2, 1, 3)                 # [c, p, q, f]
        nz = np.ascontiguousarray(
            nz.reshape(nchunks, 128, CHUNK * F), dtype=np.float32)
        in_maps.append({"wq": wq, "noise": nz, "ident_d": ident})
    return in_maps


def assemble_output(outs, T):
    """outs: per-core {"out": [128, 256]} -> [B, N] mean activity."""
    half = T // 2
    mean = np.empty((B, N), np.float32)
    for r in range(NCORES):
        oc = np.asarray(outs[r]["out"], np.float32)   # [p, jt*16 + b]
        oc = oc.reshape(128, NT, BL)                  # [p, jt, b]
        blk = oc.transpose(2, 1, 0).reshape(BL, N)    # [b, (jt, p)]
        mean[BL * r:BL * r + BL, :] = blk
    return mean / np.float32(half)


_NC_CACHE = {}


def _ensure_ntff_hook():
    """The agent image's antenv lacks axon_hooks; recreate it so
    run_bass_kernel_spmd(trace=True) can capture NTFF profiles."""
    import types
    import ctypes
    import contextlib
    try:
        from antenv.axon_hooks import get_axon_ntff_profile_hook  # noqa: F401
        return
    except ImportError:
        pass
    so_path = "/opt/axon/libaxon_pjrt.so"
    try:
        lib = ctypes.CDLL(so_path)
        if not hasattr(lib, "axon_start_nrt_profile"):
            return
    except OSError:
        return
    lib.axon_start_nrt_profile.argtypes = [
        ctypes.POINTER(ctypes.c_int64), ctypes.c_size_t]
    lib.axon_start_nrt_profile.restype = ctypes.c_int64
    lib.axon_stop_nrt_profile.argtypes = [ctypes.c_char_p]
    lib.axon_stop_nrt_profile.restype = ctypes.c_int64

    @contextlib.contextmanager
    def _hook(output_dir, device_ids):
        import jax
        jax.devices()
        if device_ids:
            ids = (ctypes.c_int64 * len(device_ids))(*device_ids)
            rc = lib.axon_start_nrt_profile(ids, len(device_ids))
        else:
            rc = lib.axon_start_nrt_profile(None, 0)
        if rc != 0:
            raise RuntimeError(f"axon_start_nrt_profile rc={rc}")
        try:
            yield
        finally:
            n = lib.axon_stop_nrt_profile(str(output_dir).encode())
            if n < 0:
                raise RuntimeError(f"axon_stop_nrt_profile rc={n}")

    mod = types.ModuleType("antenv.axon_hooks")
    mod._hook = _hook
    mod.get_axon_ntff_profile_hook = lambda: mod._hook
    mod.set_axon_ntff_profile_hook = lambda h: setattr(mod, "_hook", h)
    sys.modules["antenv.axon_hooks"] = mod


def kernel(cue, weights, noise, steps, cue_duration, trace=False):
    T = int(steps)
    cd = int(cue_duration)
    in_maps = prep_inputs(cue, weights, noise, T, cd)
    if T not in _NC_CACHE:
        nc_new = build_nc(T)
        nc_new.finalize()
        _NC_CACHE[T] = nc_new
    nc = _NC_CACHE[T]
    if trace:
        _ensure_ntff_hook()
    res = run_bass_kernel_spmd(nc, in_maps, list(range(NCORES)), trace=trace)
    out = assemble_output(res.results, T)
    kernel.last_result = res
    return out


# revision 47
# speedup vs baseline: 26.8865x; 1.2916x over previous
"""Trainium2 Bass kernel for the AttractorNetwork LIF recurrent scan.

Strategy (8 NeuronCores, one chip): pure data-parallel over batch, ZERO
cross-core communication. Each core owns 16 batch rows and keeps the full
[2048, 2048] effective weight matrix in SBUF as bf16 (8 MB). Per timestep:

  rec[b, :] = spk[b, :] @ w_eff        -> 64 matmuls (N=512, M=16), packed
                                          4-wide into PE column groups via
                                          tile_position so four j-slices
                                          compute concurrently
  rec^T chunks via 16 PE-transposes    -> back into the [neuron, batch]
                                          state layout (128 partitions)
  LIF update on VectorE (full width)   -> mem, spike, reset, activity accum

The cue is folded into the noise on the host (noise'[t<cue_d] += cue), so
the device loop is: u = mem*decay + rec + noise; spk = u >= 1;
mem = u * (u < 1); acc += spk (second half only). Spikes are stored as bf16
{0,1} (exact) and feed the next step's matmul as the stationary operand;
weights are bf16 (validated: output identical for the task's inputs).

Cores never exchange data, so there is no sensitivity to launch skew or
cross-core latency; the harness gathers per-core [128, 256] activity
accumulators and reassembles the [128, 2048] mean-activity output.
"""

import sys

sys.path.insert(0, "/opt/trn_rl_repo")

import numpy as np
import ml_dtypes

import concourse.bass as bass
import concourse.mybir as mybir
from concourse.bacc import Bacc
from concourse.bass_utils import run_bass_kernel_spmd

F32 = mybir.dt.float32
BF16 = mybir.dt.bfloat16
OP = mybir.AluOpType

N = 2048
B = 128
NCORES = 8
BL = B // NCORES         # 16 batch rows per core
NT = N // 128            # 16 neuron tiles
TAU_MEM = 20.0
DT_ = 1.0
INHIBITION = 0.1
V_THRESH = 1.0
CUE_STRENGTH = 1.0
DECAY = float(np.float32(np.exp(-DT_ / TAU_MEM)))
CHUNK = 4                # noise steps per DMA
RING = 4                 # chunks resident in the SBUF noise ring
F = NT * BL              # 256: state free width ([p, jt*16+b])


def build_nc(T, debug=False, lowering=True, drains=False):
    """Build the (SPMD but communication-free) Bass program for T steps."""
    half = T // 2
    nchunks = (T + CHUNK - 1) // CHUNK

    if lowering:
        nc = Bacc(debug=debug)
    else:
        nc = bass.Bass(debug=debug, target_bir_lowering=False)

    wq = nc.declare_dram_parameter("wq", [128, NT * N], BF16, isOutput=False)
    noise_d = nc.declare_dram_parameter(
        "noise", [nchunks, 128, CHUNK * F], F32, isOutput=False)
    ident_d = nc.declare_dram_parameter("ident_d", [128, 64], BF16, isOutput=False)
    out_d = nc.declare_dram_parameter("out", [128, F], F32, isOutput=True)

    from contextlib import ExitStack
    with ExitStack() as es:
        w_sb = es.enter_context(nc.sbuf_tensor("w_sb", [128, NT * N], BF16))
        ring = es.enter_context(
            nc.sbuf_tensor("ring", [128, RING * CHUNK * F], F32))
        rec_sb = es.enter_context(nc.sbuf_tensor("rec_sb", [128, 512], BF16))
        ident = es.enter_context(nc.sbuf_tensor("ident", [128, 64], BF16))
        zeros = es.enter_context(nc.sbuf_tensor("zeros", [128, 128], BF16))
        spk0 = es.enter_context(nc.sbuf_tensor("spk0", [128, F], BF16))
        spk1 = es.enter_context(nc.sbuf_tensor("spk1", [128, F], BF16))
        mem = es.enter_context(nc.sbuf_tensor("mem", [128, F], F32))
        u = es.enter_context(nc.sbuf_tensor("u", [128, F], F32))
        acc = es.enter_context(nc.sbuf_tensor("acc", [128, F], F32))
        psr0 = es.enter_context(nc.psum_tensor("psr0", [128, 512], F32))
        psr1 = es.enter_context(nc.psum_tensor("psr1", [128, 512], F32))
        pst0 = es.enter_context(nc.psum_tensor("pst0", [128, 512], F32))
        pst1 = es.enter_context(nc.psum_tensor("pst1", [128, 512], F32))
        w_sem = es.enter_context(nc.semaphore("w_sem"))
        noise_rdy = [
            es.enter_context(nc.semaphore(f"noise_rdy{i}")) for i in range(RING)
        ]
        noise_cons = es.enter_context(nc.semaphore("noise_cons"))
        mm_done = es.enter_context(nc.semaphore("mm_done"))
        copy_sem = es.enter_context(nc.semaphore("copy_sem"))
        tp_done = es.enter_context(nc.semaphore("tp_done"))
        spk_own = es.enter_context(nc.semaphore("spk_own"))
        u_done = es.enter_context(nc.semaphore("u_done"))
        init_done = es.enter_context(nc.semaphore("init_done"))
        odma = es.enter_context(nc.semaphore("odma"))
        block = es.enter_context(nc.Block())

        spk_buf = [spk0, spk1]
        ps_rec = [psr0, psr1]
        ps_spk = [pst0, pst1]

        def noise_ap(t):
            c = (t % (RING * CHUNK)) * F
            return ring[:, c:c + F]

        @block.sync
        def _(sync):
            sync.dma_start(out=ident[:, :], in_=ident_d[:, :]).then_inc(w_sem, 16)
            sync.dma_start(out=w_sb[:, :], in_=wq[:, :]).then_inc(w_sem, 16)
            for c in range(nchunks):
                if c >= RING:
                    sync.wait_ge(noise_cons, (c - RING) * CHUNK + CHUNK)
                s = (c % RING) * CHUNK * F
                sync.dma_start(
                    out=ring[:, s:s + CHUNK * F], in_=noise_d[c]
                ).then_inc(noise_rdy[c % RING], 16)
            sync.wait_ge(noise_cons, T)
            sync.dma_start(out=out_d[:, :], in_=acc[:, :]).then_inc(odma, 16)
            sync.wait_ge(odma, 16)

        @block.tensor
        def _(tensor):
            tensor.wait_ge(w_sem, 32)
            tensor.wait_ge(init_done, 1)
            # dummy start=True matmuls: set the has_written bits of every
            # ps_spk element once, so later sel-matmuls with start=False
            # ACCUMULATE onto DVE-written membrane values (the documented
            # cayman DVE-write + matmul-accumulate workaround)
            for p in range(2):
                # one start=True matmul covering every used column: start
                # clears the whole bank's has_written bits, so it must be a
                # single group (rhs content is irrelevant, lhsT is zeros)
                dm = tensor.matmul(
                    ps_spk[p][:, 0:F],
                    zeros[:, :],
                    spk_buf[p][:, :],
                    start=True, stop=True,
                    skip_group_check=True,
                )
            dm.then_inc(init_done, 1)
            for t in range(1, T):
                par = t % 2
                ppar = (t - 1) % 2
                tensor.wait_ge(spk_own, t)
                for i in range(NT):
                    lcol = 64 * (i % 4) + BL * (i // 4)
                    for g in range(4):
                        mm = tensor.matmul(
                            ps_rec[par][32 * g:32 * g + BL, :],
                            spk_buf[ppar][:, lcol:lcol + BL],
                            w_sb[:, N * i + 512 * g:N * i + 512 * g + 512],
                            start=(i == 0),
                            stop=(i == NT - 1),
                            tile_position=(0, 32 * g),
                            skip_group_check=True,
                        )
                mm.then_inc(mm_done, 1)
                tensor.wait_ge(copy_sem, t)
                tensor.wait_ge(u_done, t)
                # transpose the four 16-row rec bands back into the state
                # layout and ADD them onto the pre-written membrane values:
                # psum col 64c+16g+b += rec_sb[32g+b, 128c+x]
                for cc in range(4):
                    tp = tensor.matmul(
                        ps_spk[par][:, 64 * cc:64 * cc + 64],
                        rec_sb[:, 128 * cc:128 * cc + 128],
                        ident[:, :],
                        start=False, stop=True,
                        skip_group_check=True,
                    )
                tp.then_inc(tp_done, 1)

        @block.vector
        def _(vector):
            vector.memset(acc[:, :], 0.0)
            # zero the never-written partition bands of the matmul PSUM so
            # the full-width rec copy reads defined data
            vector.memset(ps_rec[0][:, :], 0.0)
            vector.memset(ps_rec[1][:, :], 0.0)
            vector.memset(spk_buf[0][:, :], 0.0)
            vector.memset(spk_buf[1][:, :], 0.0)
            vector.memset(zeros[:, :], 0.0).then_inc(init_done, 1)
            vector.wait_ge(init_done, 2)
            # t = 0: mem was 0, rec = 0 -> membrane is just noise'(0)
            vector.wait_ge(noise_rdy[0], 16)
            na = noise_ap(0)
            vector.tensor_scalar(
                spk_buf[0][:, :], na, V_THRESH, None, OP.is_ge
            ).then_inc(spk_own, 1)
            vector.scalar_tensor_tensor(
                mem[:, :], na, V_THRESH, na, OP.is_lt, OP.mult
            ).then_inc(noise_cons, 1)

            for t in range(1, T):
                par = t % 2
                if t % CHUNK == 0:
                    c = t // CHUNK
                    vector.wait_ge(noise_rdy[c % RING], 16 * (c // RING + 1))
                if drains:
                    vector.drain()   # order: mem/acc writes of step t-1
                # pre-write the leak+noise membrane into the spike PSUM; the
                # sel-matmuls accumulate rec^T on top (has_written is set)
                vector.scalar_tensor_tensor(
                    ps_spk[par][:, 0:F], mem[:, :], DECAY, noise_ap(t),
                    OP.mult, OP.add).then_inc(u_done, 1)
                vector.wait_ge(mm_done, t)
                vector.tensor_copy(
                    rec_sb[:, :], ps_rec[par][:, :]).then_inc(copy_sem, 1)
                vector.wait_ge(tp_done, t)
                vector.tensor_scalar(
                    spk_buf[par][:, :], ps_spk[par][:, 0:F],
                    V_THRESH, None, OP.is_ge
                ).then_inc(spk_own, 1)
                if drains:
                    vector.drain()   # order: spk write before mem gate read
                if t >= half:
                    vector.scalar_tensor_tensor(
                        acc[:, :], ps_spk[par][:, 0:F], V_THRESH, acc[:, :],
                        OP.is_ge, OP.add)
                # mem = u * (u < 1) == (spk == 0) * u, single PSUM read
                vector.scalar_tensor_tensor(
                    mem[:, :], spk_buf[par][:, :], 0.0,
                    ps_spk[par][:, 0:F], OP.is_equal, OP.mult
                ).then_inc(noise_cons, 1)

    return nc


def prep_inputs(cue, weights, noise, T, cue_duration):
    """Host-side sharding: returns in_maps for run_bass_kernel_spmd."""
    cue = np.asarray(cue, np.float32)
    weights = np.asarray(weights, np.float32)
    noise = np.asarray(noise, np.float32)

    w_eff = (weights - np.float32(INHIBITION / N)) * (
        1.0 - np.eye(N, dtype=np.float32))

    noise_eff = noise.copy()
    noise_eff[:cue_duration] += np.float32(CUE_STRENGTH) * cue

    nchunks = (T + CHUNK - 1) // CHUNK
    pad = nchunks * CHUNK - T
    if pad:
        noise_eff = np.concatenate(
            [noise_eff, np.zeros((pad, B, N), np.float32)], axis=0)

    # replicated weights: wq[p, i_tile*N + j] = w_eff[128*i_tile + p, j]
    wq = np.ascontiguousarray(
        w_eff.reshape(NT, 128, N).transpose(1, 0, 2).reshape(128, NT * N)
    ).astype(ml_dtypes.bfloat16)

    # 0/1 selection matrix for the rec-band transpose matmuls:
    # column 16g+b picks row 32g+b
    ident = np.zeros((128, 64), ml_dtypes.bfloat16)
    for g in range(4):
        for b in range(BL):
            ident[32 * g + b, 16 * g + b] = 1.0

    in_maps = []
    for r in range(NCORES):
        bsl = slice(BL * r, BL * r + BL)
        # noise: [t, b, j] -> [chunk, p, (q, cc, g, b)] where the state free
        # order is (cc, g, b) with neuron tile jt = 4g + cc
        nz = noise_eff[:, bsl, :]                     # [Tp, 16, 2048]
        nz = nz.transpose(0, 2, 1)                    # [Tp, 2048j, 16b]
        nz = nz.reshape(-1, 4, 4, 128, BL)            # [Tp, g, cc, p, b]
        nz = nz.transpose(0, 3, 2, 1, 4)              # [Tp, p, cc, g, b]
        nz = nz.reshape(nchunks, CHUNK, 128, F)       # [c, q, p, f]
        nz = nz.transpose(0, 2, 1, 3)                 # [c, p, q, f]
        nz = np.ascontiguousarray(
            nz.reshape(nchunks, 128, CHUNK * F), dtype=np.float32)
        in_maps.append({"wq": wq, "noise": nz, "ident_d": ident})
    return in_maps


def assemble_output(outs, T):
    """outs: per-core {"out": [128, 256]} -> [B, N] mean activity."""
    half = T // 2
    mean = np.empty((B, N), np.float32)
    for r in range(NCORES):
        oc = np.asarray(outs[r]["out"], np.float32)   # [p, 64cc+16g+b]
        oc = oc.reshape(128, 4, 4, BL)                # [p, cc, g, b]
        blk = oc.transpose(3, 2, 1, 0).reshape(BL, N)  # [b, (g, cc, p)]
        mean[BL * r:BL * r + BL, :] = blk
    return mean / np.float32(half)


_NC_CACHE = {}


def _ensure_ntff_hook():
    """The agent image's antenv lacks axon_hooks; recreate it so
    run_bass_kernel_spmd(trace=True) can capture NTFF profiles."""
    import types
    import ctypes
    import contextlib
    try:
        from antenv.axon_hooks import get_axon_ntff_profile_hook  # noqa: F401
        return
    except ImportError:
        pass
    so_path = "/opt/axon/libaxon_pjrt.so"
    try:
        lib = ctypes.CDLL(so_path)
        if not hasattr(lib, "axon_start_nrt_profile"):
            return
    except OSError:
        return
    lib.axon_start_nrt_profile.argtypes = [
        ctypes.POINTER(ctypes.c_int64), ctypes.c_size_t]
    lib.axon_start_nrt_profile.restype = ctypes.c_int64
    lib.axon_stop_nrt_profile.argtypes = [ctypes.c_char_p]
    lib.axon_stop_nrt_profile.restype = ctypes.c_int64

    @contextlib.contextmanager
    def _hook(output_dir, device_ids):
        import jax
        jax.devices()
        if device_ids:
            ids = (ctypes.c_int64 * len(device_ids))(*device_ids)
            rc = lib.axon_start_nrt_profile(ids, len(device_ids))
        else:
            rc = lib.axon_start_nrt_profile(None, 0)
        if rc != 0:
            raise RuntimeError(f"axon_start_nrt_profile rc={rc}")
        try:
            yield
        finally:
            n = lib.axon_stop_nrt_profile(str(output_dir).encode())
            if n < 0:
                raise RuntimeError(f"axon_stop_nrt_profile rc={n}")

    mod = types.ModuleType("antenv.axon_hooks")
    mod._hook = _hook
    mod.get_axon_ntff_profile_hook = lambda: mod._hook
    mod.set_axon_ntff_profile_hook = lambda h: setattr(mod, "_hook", h)
    sys.modules["antenv.axon_hooks"] = mod


def kernel(cue, weights, noise, steps, cue_duration, trace=False):
    T = int(steps)
    cd = int(cue_duration)
    in_maps = prep_inputs(cue, weights, noise, T, cd)
    if T not in _NC_CACHE:
        nc_new = build_nc(T)
        nc_new.finalize()
        _NC_CACHE[T] = nc_new
    nc = _NC_CACHE[T]
    if trace:
        _ensure_ntff_hook()
    res = run_bass_kernel_spmd(nc, in_maps, list(range(NCORES)), trace=trace)
    out = assemble_output(res.results, T)
    kernel.last_result = res
    return out


# revision 50
# speedup vs baseline: 27.2217x; 1.0125x over previous
"""Trainium2 Bass kernel for the AttractorNetwork LIF recurrent scan.

Strategy (8 NeuronCores, one chip): pure data-parallel over batch, ZERO
cross-core communication. Each core owns 16 batch rows and keeps the full
[2048, 2048] effective weight matrix in SBUF as bf16 (8 MB). Per timestep:

  rec[b, :] = spk[b, :] @ w_eff        -> 64 matmuls (N=512, M=16), packed
                                          4-wide into PE column groups via
                                          tile_position so four j-slices
                                          compute concurrently
  rec^T chunks via 16 PE-transposes    -> back into the [neuron, batch]
                                          state layout (128 partitions)
  LIF update on VectorE (full width)   -> mem, spike, reset, activity accum

The cue is folded into the noise on the host (noise'[t<cue_d] += cue), so
the device loop is: u = mem*decay + rec + noise; spk = u >= 1;
mem = u * (u < 1); acc += spk (second half only). Spikes are stored as bf16
{0,1} (exact) and feed the next step's matmul as the stationary operand;
weights are bf16 (validated: output identical for the task's inputs).

Cores never exchange data, so there is no sensitivity to launch skew or
cross-core latency; the harness gathers per-core [128, 256] activity
accumulators and reassembles the [128, 2048] mean-activity output.
"""

import sys

sys.path.insert(0, "/opt/trn_rl_repo")

import numpy as np
import ml_dtypes

import concourse.bass as bass
import concourse.mybir as mybir
from concourse.bacc import Bacc
from concourse.bass_utils import run_bass_kernel_spmd

F32 = mybir.dt.float32
BF16 = mybir.dt.bfloat16
OP = mybir.AluOpType

N = 2048
B = 128
NCORES = 8
BL = B // NCORES         # 16 batch rows per core
NT = N // 128            # 16 neuron tiles
TAU_MEM = 20.0
DT_ = 1.0
INHIBITION = 0.1
V_THRESH = 1.0
CUE_STRENGTH = 1.0
DECAY = float(np.float32(np.exp(-DT_ / TAU_MEM)))
CHUNK = 4                # noise steps per DMA
RING = 4                 # chunks resident in the SBUF noise ring
F = NT * BL              # 256: state free width ([p, jt*16+b])


def build_nc(T, debug=False, lowering=True, drains=False):
    """Build the (SPMD but communication-free) Bass program for T steps."""
    half = T // 2
    nchunks = (T + CHUNK - 1) // CHUNK

    if lowering:
        nc = Bacc(debug=debug)
    else:
        nc = bass.Bass(debug=debug, target_bir_lowering=False)

    wq = nc.declare_dram_parameter("wq", [128, NT * N], BF16, isOutput=False)
    noise_d = nc.declare_dram_parameter(
        "noise", [nchunks, 128, CHUNK * F], F32, isOutput=False)
    ident_d = nc.declare_dram_parameter("ident_d", [128, 64], BF16, isOutput=False)
    out_d = nc.declare_dram_parameter("out", [128, F], F32, isOutput=True)

    from contextlib import ExitStack
    with ExitStack() as es:
        w_sb = es.enter_context(nc.sbuf_tensor("w_sb", [128, NT * N], BF16))
        ring = es.enter_context(
            nc.sbuf_tensor("ring", [128, RING * CHUNK * F], F32))
        rec_sb = es.enter_context(nc.sbuf_tensor("rec_sb", [128, 512], BF16))
        ident = es.enter_context(nc.sbuf_tensor("ident", [128, 64], BF16))
        zeros = es.enter_context(nc.sbuf_tensor("zeros", [128, 128], BF16))
        spk0 = es.enter_context(nc.sbuf_tensor("spk0", [128, F], BF16))
        spk1 = es.enter_context(nc.sbuf_tensor("spk1", [128, F], BF16))
        mem = es.enter_context(nc.sbuf_tensor("mem", [128, F], F32))
        u = es.enter_context(nc.sbuf_tensor("u", [128, F], F32))
        acc = es.enter_context(nc.sbuf_tensor("acc", [128, F], F32))
        psr0 = es.enter_context(nc.psum_tensor("psr0", [128, 512], F32))
        psr1 = es.enter_context(nc.psum_tensor("psr1", [128, 512], F32))
        pst0 = es.enter_context(nc.psum_tensor("pst0", [128, 512], F32))
        pst1 = es.enter_context(nc.psum_tensor("pst1", [128, 512], F32))
        w_sem = es.enter_context(nc.semaphore("w_sem"))
        noise_rdy = [
            es.enter_context(nc.semaphore(f"noise_rdy{i}")) for i in range(RING)
        ]
        noise_cons = es.enter_context(nc.semaphore("noise_cons"))
        mm_done = es.enter_context(nc.semaphore("mm_done"))
        copy_sem = es.enter_context(nc.semaphore("copy_sem"))
        tp_done = es.enter_context(nc.semaphore("tp_done"))
        spk_own = es.enter_context(nc.semaphore("spk_own"))
        u_done = es.enter_context(nc.semaphore("u_done"))
        init_done = es.enter_context(nc.semaphore("init_done"))
        odma = es.enter_context(nc.semaphore("odma"))
        block = es.enter_context(nc.Block())

        spk_buf = [spk0, spk1]
        ps_rec = [psr0, psr1]
        ps_spk = [pst0, pst1]

        def noise_ap(t):
            c = (t % (RING * CHUNK)) * F
            return ring[:, c:c + F]

        @block.sync
        def _(sync):
            sync.dma_start(out=ident[:, :], in_=ident_d[:, :]).then_inc(w_sem, 16)
            sync.dma_start(out=w_sb[:, :], in_=wq[:, :]).then_inc(w_sem, 16)
            for c in range(nchunks):
                if c >= RING:
                    sync.wait_ge(noise_cons, (c - RING) * CHUNK + CHUNK)
                s = (c % RING) * CHUNK * F
                sync.dma_start(
                    out=ring[:, s:s + CHUNK * F], in_=noise_d[c]
                ).then_inc(noise_rdy[c % RING], 16)
            sync.wait_ge(noise_cons, T)
            sync.dma_start(out=out_d[:, :], in_=acc[:, :]).then_inc(odma, 16)
            sync.wait_ge(odma, 16)

        @block.tensor
        def _(tensor):
            tensor.wait_ge(w_sem, 32)
            tensor.wait_ge(init_done, 1)
            # dummy start=True matmuls: set the has_written bits of every
            # ps_spk element once, so later sel-matmuls with start=False
            # ACCUMULATE onto DVE-written membrane values (the documented
            # cayman DVE-write + matmul-accumulate workaround)
            for p in range(2):
                # one start=True matmul covering every used column: start
                # clears the whole bank's has_written bits, so it must be a
                # single group (rhs content is irrelevant, lhsT is zeros)
                dm = tensor.matmul(
                    ps_spk[p][:, 0:F],
                    zeros[:, :],
                    spk_buf[p][:, :],
                    start=True, stop=True,
                    skip_group_check=True,
                )
            dm.then_inc(init_done, 1)
            for t in range(1, T):
                par = t % 2
                ppar = (t - 1) % 2
                for i in range(NT):
                    if i < 4:
                        # spike chunk i of step t-1 (is_ge is split 4-ways)
                        tensor.wait_ge(spk_own, 4 * (t - 1) + i + 1)
                    lcol = 64 * (i % 4) + BL * (i // 4)
                    for g in range(4):
                        mm = tensor.matmul(
                            ps_rec[par][32 * g:32 * g + BL, :],
                            spk_buf[ppar][:, lcol:lcol + BL],
                            w_sb[:, N * i + 512 * g:N * i + 512 * g + 512],
                            start=(i == 0),
                            stop=(i == NT - 1),
                            tile_position=(0, 32 * g),
                            skip_group_check=True,
                        )
                mm.then_inc(mm_done, 1)
                tensor.wait_ge(u_done, t)
                # transpose the four 16-row rec bands back into the state
                # layout and ADD them onto the pre-written membrane values:
                # psum col 64c+16g+b += rec_sb[32g+b, 128c+x]
                for cc in range(4):
                    tensor.wait_ge(copy_sem, 4 * (t - 1) + cc + 1)
                    tp = tensor.matmul(
                        ps_spk[par][:, 64 * cc:64 * cc + 64],
                        rec_sb[:, 128 * cc:128 * cc + 128],
                        ident[:, :],
                        start=False, stop=True,
                        skip_group_check=True,
                    )
                tp.then_inc(tp_done, 1)

        @block.vector
        def _(vector):
            vector.memset(acc[:, :], 0.0)
            # zero the never-written partition bands of the matmul PSUM so
            # the full-width rec copy reads defined data
            vector.memset(ps_rec[0][:, :], 0.0)
            vector.memset(ps_rec[1][:, :], 0.0)
            vector.memset(spk_buf[0][:, :], 0.0)
            vector.memset(spk_buf[1][:, :], 0.0)
            vector.memset(zeros[:, :], 0.0).then_inc(init_done, 1)
            vector.wait_ge(init_done, 2)
            # t = 0: mem was 0, rec = 0 -> membrane is just noise'(0)
            vector.wait_ge(noise_rdy[0], 16)
            na = noise_ap(0)
            vector.tensor_scalar(
                spk_buf[0][:, :], na, V_THRESH, None, OP.is_ge
            ).then_inc(spk_own, 4)
            vector.scalar_tensor_tensor(
                mem[:, :], na, V_THRESH, na, OP.is_lt, OP.mult
            ).then_inc(noise_cons, 1)

            for t in range(1, T):
                par = t % 2
                if t % CHUNK == 0:
                    c = t // CHUNK
                    vector.wait_ge(noise_rdy[c % RING], 16 * (c // RING + 1))
                if drains:
                    vector.drain()   # order: mem/acc writes of step t-1
                # pre-write the leak+noise membrane into the spike PSUM; the
                # sel-matmuls accumulate rec^T on top (has_written is set)
                vector.scalar_tensor_tensor(
                    ps_spk[par][:, 0:F], mem[:, :], DECAY, noise_ap(t),
                    OP.mult, OP.add).then_inc(u_done, 1)
                vector.wait_ge(mm_done, t)
                for cc in range(4):
                    # rec copy+cast chunk cc feeds sel-matmul chunk cc
                    vector.tensor_copy(
                        rec_sb[:, 128 * cc:128 * cc + 128],
                        ps_rec[par][:, 128 * cc:128 * cc + 128],
                    ).then_inc(copy_sem, 1)
                vector.wait_ge(tp_done, t)
                for cc in range(4):
                    # spike chunk cc unblocks next step's matmul wave cc
                    vector.tensor_scalar(
                        spk_buf[par][:, 64 * cc:64 * cc + 64],
                        ps_spk[par][:, 64 * cc:64 * cc + 64],
                        V_THRESH, None, OP.is_ge
                    ).then_inc(spk_own, 1)
                if drains:
                    vector.drain()   # order: spk write before mem gate read
                if t >= half:
                    vector.scalar_tensor_tensor(
                        acc[:, :], ps_spk[par][:, 0:F], V_THRESH, acc[:, :],
                        OP.is_ge, OP.add)
                # mem = u * (u < 1) == (spk == 0) * u, single PSUM read
                vector.scalar_tensor_tensor(
                    mem[:, :], spk_buf[par][:, :], 0.0,
                    ps_spk[par][:, 0:F], OP.is_equal, OP.mult
                ).then_inc(noise_cons, 1)

    return nc


def prep_inputs(cue, weights, noise, T, cue_duration):
    """Host-side sharding: returns in_maps for run_bass_kernel_spmd."""
    cue = np.asarray(cue, np.float32)
    weights = np.asarray(weights, np.float32)
    noise = np.asarray(noise, np.float32)

    w_eff = (weights - np.float32(INHIBITION / N)) * (
        1.0 - np.eye(N, dtype=np.float32))

    noise_eff = noise.copy()
    noise_eff[:cue_duration] += np.float32(CUE_STRENGTH) * cue

    nchunks = (T + CHUNK - 1) // CHUNK
    pad = nchunks * CHUNK - T
    if pad:
        noise_eff = np.concatenate(
            [noise_eff, np.zeros((pad, B, N), np.float32)], axis=0)

    # replicated weights: wq[p, i_tile*N + j] = w_eff[128*i_tile + p, j]
    wq = np.ascontiguousarray(
        w_eff.reshape(NT, 128, N).transpose(1, 0, 2).reshape(128, NT * N)
    ).astype(ml_dtypes.bfloat16)

    # 0/1 selection matrix for the rec-band transpose matmuls:
    # column 16g+b picks row 32g+b
    ident = np.zeros((128, 64), ml_dtypes.bfloat16)
    for g in range(4):
        for b in range(BL):
            ident[32 * g + b, 16 * g + b] = 1.0

    in_maps = []
    for r in range(NCORES):
        bsl = slice(BL * r, BL * r + BL)
        # noise: [t, b, j] -> [chunk, p, (q, cc, g, b)] where the state free
        # order is (cc, g, b) with neuron tile jt = 4g + cc
        nz = noise_eff[:, bsl, :]                     # [Tp, 16, 2048]
        nz = nz.transpose(0, 2, 1)                    # [Tp, 2048j, 16b]
        nz = nz.reshape(-1, 4, 4, 128, BL)            # [Tp, g, cc, p, b]
        nz = nz.transpose(0, 3, 2, 1, 4)              # [Tp, p, cc, g, b]
        nz = nz.reshape(nchunks, CHUNK, 128, F)       # [c, q, p, f]
        nz = nz.transpose(0, 2, 1, 3)                 # [c, p, q, f]
        nz = np.ascontiguousarray(
            nz.reshape(nchunks, 128, CHUNK * F), dtype=np.float32)
        in_maps.append({"wq": wq, "noise": nz, "ident_d": ident})
    return in_maps


def assemble_output(outs, T):
    """outs: per-core {"out": [128, 256]} -> [B, N] mean activity."""
    half = T // 2
    mean = np.empty((B, N), np.float32)
    for r in range(NCORES):
        oc = np.asarray(outs[r]["out"], np.float32)   # [p, 64cc+16g+b]
        oc = oc.reshape(128, 4, 4, BL)                # [p, cc, g, b]
        blk = oc.transpose(3, 2, 1, 0).reshape(BL, N)  # [b, (g, cc, p)]
        mean[BL * r:BL * r + BL, :] = blk
    return mean / np.float32(half)


_NC_CACHE = {}


def _ensure_ntff_hook():
    """The agent image's antenv lacks axon_hooks; recreate it so
    run_bass_kernel_spmd(trace=True) can capture NTFF profiles."""
    import types
    import ctypes
    import contextlib
    try:
        from antenv.axon_hooks import get_axon_ntff_profile_hook  # noqa: F401
        return
    except ImportError:
        pass
    so_path = "/opt/axon/libaxon_pjrt.so"
    try:
        lib = ctypes.CDLL(so_path)
        if not hasattr(lib, "axon_start_nrt_profile"):
            return
    except OSError:
        return
    lib.axon_start_nrt_profile.argtypes = [
        ctypes.POINTER(ctypes.c_int64), ctypes.c_size_t]
    lib.axon_start_nrt_profile.restype = ctypes.c_int64
    lib.axon_stop_nrt_profile.argtypes = [ctypes.c_char_p]
    lib.axon_stop_nrt_profile.restype = ctypes.c_int64

    @contextlib.contextmanager
    def _hook(output_dir, device_ids):
        import jax
        jax.devices()
        if device_ids:
            ids = (ctypes.c_int64 * len(device_ids))(*device_ids)
            rc = lib.axon_start_nrt_profile(ids, len(device_ids))
        else:
            rc = lib.axon_start_nrt_profile(None, 0)
        if rc != 0:
            raise RuntimeError(f"axon_start_nrt_profile rc={rc}")
        try:
            yield
        finally:
            n = lib.axon_stop_nrt_profile(str(output_dir).encode())
            if n < 0:
                raise RuntimeError(f"axon_stop_nrt_profile rc={n}")

    mod = types.ModuleType("antenv.axon_hooks")
    mod._hook = _hook
    mod.get_axon_ntff_profile_hook = lambda: mod._hook
    mod.set_axon_ntff_profile_hook = lambda h: setattr(mod, "_hook", h)
    sys.modules["antenv.axon_hooks"] = mod


def kernel(cue, weights, noise, steps, cue_duration, trace=False):
    T = int(steps)
    cd = int(cue_duration)
    in_maps = prep_inputs(cue, weights, noise, T, cd)
    if T not in _NC_CACHE:
        nc_new = build_nc(T)
        nc_new.finalize()
        _NC_CACHE[T] = nc_new
    nc = _NC_CACHE[T]
    if trace:
        _ensure_ntff_hook()
    res = run_bass_kernel_spmd(nc, in_maps, list(range(NCORES)), trace=trace)
    out = assemble_output(res.results, T)
    kernel.last_result = res
    return out


# revision 53
# speedup vs baseline: 30.2738x; 1.1121x over previous
"""Trainium2 Bass kernel for the AttractorNetwork LIF recurrent scan.

Strategy (8 NeuronCores, one chip): pure data-parallel over batch, ZERO
cross-core communication. Each core owns 16 batch rows and keeps the full
[2048, 2048] effective weight matrix in SBUF as bf16 (8 MB). Per timestep:

  rec[b, :] = spk[b, :] @ w_eff        -> 64 matmuls (N=512, M=16), packed
                                          4-wide into PE column groups via
                                          tile_position so four j-slices
                                          compute concurrently
  rec^T chunks via 16 PE-transposes    -> back into the [neuron, batch]
                                          state layout (128 partitions)
  LIF update on VectorE (full width)   -> mem, spike, reset, activity accum

The cue is folded into the noise on the host (noise'[t<cue_d] += cue), so
the device loop is: u = mem*decay + rec + noise; spk = u >= 1;
mem = u * (u < 1); acc += spk (second half only). Spikes are stored as bf16
{0,1} (exact) and feed the next step's matmul as the stationary operand;
weights are bf16 (validated: output identical for the task's inputs).

Cores never exchange data, so there is no sensitivity to launch skew or
cross-core latency; the harness gathers per-core [128, 256] activity
accumulators and reassembles the [128, 2048] mean-activity output.
"""

import sys

sys.path.insert(0, "/opt/trn_rl_repo")

import numpy as np
import ml_dtypes

import concourse.bass as bass
import concourse.mybir as mybir
from concourse.bacc import Bacc
from concourse.bass_utils import run_bass_kernel_spmd

F32 = mybir.dt.float32
BF16 = mybir.dt.bfloat16
OP = mybir.AluOpType

N = 2048
B = 128
NCORES = 8
BL = B // NCORES         # 16 batch rows per core
NT = N // 128            # 16 neuron tiles
TAU_MEM = 20.0
DT_ = 1.0
INHIBITION = 0.1
V_THRESH = 1.0
CUE_STRENGTH = 1.0
DECAY = float(np.float32(np.exp(-DT_ / TAU_MEM)))
CHUNK = 4                # noise steps per DMA
RING = 4                 # chunks resident in the SBUF noise ring
F = NT * BL              # 256: state free width ([p, jt*16+b])


def build_nc(T, debug=False, lowering=True, drains=False):
    """Build the (SPMD but communication-free) Bass program for T steps."""
    half = T // 2
    nchunks = (T + CHUNK - 1) // CHUNK

    if lowering:
        nc = Bacc(debug=debug)
    else:
        nc = bass.Bass(debug=debug, target_bir_lowering=False)

    wq = nc.declare_dram_parameter("wq", [128, NT * N], BF16, isOutput=False)
    noise_d = nc.declare_dram_parameter(
        "noise", [nchunks, 128, CHUNK * F], F32, isOutput=False)
    ident_d = nc.declare_dram_parameter("ident_d", [128, 64], BF16, isOutput=False)
    out_d = nc.declare_dram_parameter("out", [128, F], F32, isOutput=True)

    from contextlib import ExitStack
    with ExitStack() as es:
        w_sb = es.enter_context(nc.sbuf_tensor("w_sb", [128, NT * N], BF16))
        ring = es.enter_context(
            nc.sbuf_tensor("ring", [128, RING * CHUNK * F], F32))
        rec_sb = es.enter_context(nc.sbuf_tensor("rec_sb", [128, 512], BF16))
        ident = es.enter_context(nc.sbuf_tensor("ident", [128, 64], BF16))
        zeros = es.enter_context(nc.sbuf_tensor("zeros", [128, 128], BF16))
        spk0 = es.enter_context(nc.sbuf_tensor("spk0", [128, F], BF16))
        spk1 = es.enter_context(nc.sbuf_tensor("spk1", [128, F], BF16))
        mem = es.enter_context(nc.sbuf_tensor("mem", [128, F], F32))
        u = es.enter_context(nc.sbuf_tensor("u", [128, F], F32))
        acc = es.enter_context(nc.sbuf_tensor("acc", [128, F], F32))
        # 8 full PSUM banks: rec[parity][half] + spk[parity][pair]; full-
        # bank allocation keeps concurrent PE-writes and DVE-reads in
        # different banks (same-bank PE-W + DVE-R is a hard fault)
        psrec_t = [[es.enter_context(
            nc.psum_tensor(f"psr{p}{h}", [128, 512], F32))
            for h in range(2)] for p in range(2)]
        psspk_t = [[es.enter_context(
            nc.psum_tensor(f"pss{p}{h}", [128, 512], F32))
            for h in range(2)] for p in range(2)]
        w_sem = es.enter_context(nc.semaphore("w_sem"))
        noise_rdy = [
            es.enter_context(nc.semaphore(f"noise_rdy{i}")) for i in range(RING)
        ]
        noise_cons = es.enter_context(nc.semaphore("noise_cons"))
        mm_done = es.enter_context(nc.semaphore("mm_done"))
        copy_sem = es.enter_context(nc.semaphore("copy_sem"))
        tp_done = es.enter_context(nc.semaphore("tp_done"))
        spk_own = es.enter_context(nc.semaphore("spk_own"))
        u_done = es.enter_context(nc.semaphore("u_done"))
        init_done = es.enter_context(nc.semaphore("init_done"))
        odma = es.enter_context(nc.semaphore("odma"))
        block = es.enter_context(nc.Block())

        spk_buf = [spk0, spk1]

        def noise_ap(t):
            c = (t % (RING * CHUNK)) * F
            return ring[:, c:c + F]

        @block.sync
        def _(sync):
            sync.dma_start(out=ident[:, :], in_=ident_d[:, :]).then_inc(w_sem, 16)
            sync.dma_start(out=w_sb[:, :], in_=wq[:, :]).then_inc(w_sem, 16)
            for c in range(nchunks):
                if c >= RING:
                    sync.wait_ge(noise_cons, (c - RING) * CHUNK + CHUNK)
                s = (c % RING) * CHUNK * F
                sync.dma_start(
                    out=ring[:, s:s + CHUNK * F], in_=noise_d[c]
                ).then_inc(noise_rdy[c % RING], 16)
            sync.wait_ge(noise_cons, T)
            sync.dma_start(out=out_d[:, :], in_=acc[:, :]).then_inc(odma, 16)
            sync.wait_ge(odma, 16)

        @block.tensor
        def _(tensor):
            tensor.wait_ge(w_sem, 32)
            tensor.wait_ge(init_done, 1)
            # dummy start=True matmuls: set the has_written bits of every
            # used ps_spk element once, so later sel-matmuls with
            # start=False ACCUMULATE onto DVE-written membrane values (the
            # documented cayman DVE-write + matmul-accumulate workaround).
            # One single-group matmul per bank (start clears the whole
            # bank's bits).
            for p in range(2):
                for pr in range(2):
                    dm = tensor.matmul(
                        psspk_t[p][pr][:, 0:128],
                        zeros[:, :],
                        spk_buf[p][:, 0:128],
                        start=True, stop=True,
                        skip_group_check=True,
                    )
            dm.then_inc(init_done, 1)
            for t in range(1, T):
                par = t % 2
                ppar = (t - 1) % 2
                # main matmul in two 256-column halves (separate banks): the
                # left half of rec is final at mid-matmul so the cast/sel/
                # is_ge tail pipelines underneath the right half
                for h2 in range(2):
                    for i in range(NT):
                        if h2 == 0 and i < 4:
                            # spike chunk i of step t-1 (is_ge split 4-ways)
                            tensor.wait_ge(spk_own, 4 * (t - 1) + i + 1)
                        lcol = 64 * (i % 4) + BL * (i // 4)
                        wcol = N * i + 256 * h2
                        for g in range(4):
                            mm = tensor.matmul(
                                psrec_t[par][h2][32 * g:32 * g + BL, 0:256],
                                spk_buf[ppar][:, lcol:lcol + BL],
                                w_sb[:, wcol + 512 * g:wcol + 512 * g + 256],
                                start=(i == 0),
                                stop=(i == NT - 1),
                                tile_position=(0, 32 * g),
                                skip_group_check=True,
                            )
                    mm.then_inc(mm_done, 1)
                # transpose the four 16-row rec bands back into the state
                # layout and ADD them onto the pre-written membrane values:
                # spk-psum pair cc//2 col 64*(cc%2)+16g+b += rec chunk cc
                for cc in range(4):
                    tensor.wait_ge(copy_sem, 4 * (t - 1) + cc + 1)
                    if cc % 2 == 0:
                        tensor.wait_ge(u_done, 2 * (t - 1) + cc // 2 + 1)
                    tp = tensor.matmul(
                        psspk_t[par][cc // 2][:, 64 * (cc % 2):
                                              64 * (cc % 2) + 64],
                        rec_sb[:, 128 * cc:128 * cc + 128],
                        ident[:, :],
                        start=False, stop=True,
                        skip_group_check=True,
                    )
                    if cc % 2 == 1:
                        tp.then_inc(tp_done, 1)

        @block.vector
        def _(vector):
            vector.memset(acc[:, :], 0.0)
            # zero the never-written partition bands of the matmul PSUM so
            # the rec copy reads defined data
            for p in range(2):
                for h in range(2):
                    vector.memset(psrec_t[p][h][:, :], 0.0)
            vector.memset(spk_buf[0][:, :], 0.0)
            vector.memset(spk_buf[1][:, :], 0.0)
            vector.memset(zeros[:, :], 0.0).then_inc(init_done, 1)
            vector.wait_ge(init_done, 2)
            # t = 0: mem was 0, rec = 0 -> membrane is just noise'(0)
            vector.wait_ge(noise_rdy[0], 16)
            na = noise_ap(0)
            vector.tensor_scalar(
                spk_buf[0][:, :], na, V_THRESH, None, OP.is_ge
            ).then_inc(spk_own, 4)
            vector.scalar_tensor_tensor(
                mem[:, :], na, V_THRESH, na, OP.is_lt, OP.mult
            ).then_inc(noise_cons, 1)

            for t in range(1, T):
                par = t % 2
                if t % CHUNK == 0:
                    c = t // CHUNK
                    vector.wait_ge(noise_rdy[c % RING], 16 * (c // RING + 1))
                if drains:
                    vector.drain()   # order: mem/acc writes of step t-1
                # pre-write the leak+noise membrane into the spike PSUM
                # pairs; the sel-matmuls accumulate rec^T on top
                for pr in range(2):
                    ns = noise_ap(t)[:, 128 * pr:128 * pr + 128]
                    vector.scalar_tensor_tensor(
                        psspk_t[par][pr][:, 0:128],
                        mem[:, 128 * pr:128 * pr + 128], DECAY, ns,
                        OP.mult, OP.add).then_inc(u_done, 1)
                for cc in range(4):
                    if cc % 2 == 0:
                        # chunks 0-1 need the left matmul half, 2-3 right
                        vector.wait_ge(mm_done, 2 * (t - 1) + cc // 2 + 1)
                    # rec copy+cast chunk cc feeds sel-matmul chunk cc
                    vector.tensor_copy(
                        rec_sb[:, 128 * cc:128 * cc + 128],
                        psrec_t[par][cc // 2][:, 128 * (cc % 2):
                                              128 * (cc % 2) + 128],
                    ).then_inc(copy_sem, 1)
                for cc in range(4):
                    # spike chunk cc unblocks next step's matmul wave cc
                    if cc % 2 == 0:
                        vector.wait_ge(tp_done, 2 * (t - 1) + cc // 2 + 1)
                    vector.tensor_scalar(
                        spk_buf[par][:, 64 * cc:64 * cc + 64],
                        psspk_t[par][cc // 2][:, 64 * (cc % 2):
                                              64 * (cc % 2) + 64],
                        V_THRESH, None, OP.is_ge
                    ).then_inc(spk_own, 1)
                if drains:
                    vector.drain()   # order: spk write before mem gate read
                for pr in range(2):
                    if t >= half:
                        vector.scalar_tensor_tensor(
                            acc[:, 128 * pr:128 * pr + 128],
                            psspk_t[par][pr][:, 0:128], V_THRESH,
                            acc[:, 128 * pr:128 * pr + 128],
                            OP.is_ge, OP.add)
                    # mem = u * (u < 1) == (spk == 0) * u, one PSUM read
                    st = vector.scalar_tensor_tensor(
                        mem[:, 128 * pr:128 * pr + 128],
                        spk_buf[par][:, 128 * pr:128 * pr + 128], 0.0,
                        psspk_t[par][pr][:, 0:128], OP.is_equal, OP.mult)
                st.then_inc(noise_cons, 1)

    return nc


def prep_inputs(cue, weights, noise, T, cue_duration):
    """Host-side sharding: returns in_maps for run_bass_kernel_spmd."""
    cue = np.asarray(cue, np.float32)
    weights = np.asarray(weights, np.float32)
    noise = np.asarray(noise, np.float32)

    w_eff = (weights - np.float32(INHIBITION / N)) * (
        1.0 - np.eye(N, dtype=np.float32))

    noise_eff = noise.copy()
    noise_eff[:cue_duration] += np.float32(CUE_STRENGTH) * cue

    nchunks = (T + CHUNK - 1) // CHUNK
    pad = nchunks * CHUNK - T
    if pad:
        noise_eff = np.concatenate(
            [noise_eff, np.zeros((pad, B, N), np.float32)], axis=0)

    # replicated weights: wq[p, i_tile*N + j] = w_eff[128*i_tile + p, j]
    wq = np.ascontiguousarray(
        w_eff.reshape(NT, 128, N).transpose(1, 0, 2).reshape(128, NT * N)
    ).astype(ml_dtypes.bfloat16)

    # 0/1 selection matrix for the rec-band transpose matmuls:
    # column 16g+b picks row 32g+b
    ident = np.zeros((128, 64), ml_dtypes.bfloat16)
    for g in range(4):
        for b in range(BL):
            ident[32 * g + b, 16 * g + b] = 1.0

    in_maps = []
    for r in range(NCORES):
        bsl = slice(BL * r, BL * r + BL)
        # noise: [t, b, j] -> [chunk, p, (q, cc, g, b)] where the state free
        # order is (cc, g, b) with neuron tile jt = 4g + cc
        nz = noise_eff[:, bsl, :]                     # [Tp, 16, 2048]
        nz = nz.transpose(0, 2, 1)                    # [Tp, 2048j, 16b]
        nz = nz.reshape(-1, 4, 4, 128, BL)            # [Tp, g, cc, p, b]
        nz = nz.transpose(0, 3, 2, 1, 4)              # [Tp, p, cc, g, b]
        nz = nz.reshape(nchunks, CHUNK, 128, F)       # [c, q, p, f]
        nz = nz.transpose(0, 2, 1, 3)                 # [c, p, q, f]
        nz = np.ascontiguousarray(
            nz.reshape(nchunks, 128, CHUNK * F), dtype=np.float32)
        in_maps.append({"wq": wq, "noise": nz, "ident_d": ident})
    return in_maps


def assemble_output(outs, T):
    """outs: per-core {"out": [128, 256]} -> [B, N] mean activity."""
    half = T // 2
    mean = np.empty((B, N), np.float32)
    for r in range(NCORES):
        oc = np.asarray(outs[r]["out"], np.float32)   # [p, 64cc+16g+b]
        oc = oc.reshape(128, 4, 4, BL)                # [p, cc, g, b]
        blk = oc.transpose(3, 2, 1, 0).reshape(BL, N)  # [b, (g, cc, p)]
        mean[BL * r:BL * r + BL, :] = blk
    return mean / np.float32(half)


_NC_CACHE = {}


def _ensure_ntff_hook():
    """The agent image's antenv lacks axon_hooks; recreate it so
    run_bass_kernel_spmd(trace=True) can capture NTFF profiles."""
    import types
    import ctypes
    import contextlib
    try:
        from antenv.axon_hooks import get_axon_ntff_profile_hook  # noqa: F401
        return
    except ImportError:
        pass
    so_path = "/opt/axon/libaxon_pjrt.so"
    try:
        lib = ctypes.CDLL(so_path)
        if not hasattr(lib, "axon_start_nrt_profile"):
            return
    except OSError:
        return
    lib.axon_start_nrt_profile.argtypes = [
        ctypes.POINTER(ctypes.c_int64), ctypes.c_size_t]
    lib.axon_start_nrt_profile.restype = ctypes.c_int64
    lib.axon_stop_nrt_profile.argtypes = [ctypes.c_char_p]
    lib.axon_stop_nrt_profile.restype = ctypes.c_int64

    @contextlib.contextmanager
    def _hook(output_dir, device_ids):
        import jax
        jax.devices()
        if device_ids:
            ids = (ctypes.c_int64 * len(device_ids))(*device_ids)
            rc = lib.axon_start_nrt_profile(ids, len(device_ids))
        else:
            rc = lib.axon_start_nrt_profile(None, 0)
        if rc != 0:
            raise RuntimeError(f"axon_start_nrt_profile rc={rc}")
        try:
            yield
        finally:
            n = lib.axon_stop_nrt_profile(str(output_dir).encode())
            if n < 0:
                raise RuntimeError(f"axon_stop_nrt_profile rc={n}")

    mod = types.ModuleType("antenv.axon_hooks")
    mod._hook = _hook
    mod.get_axon_ntff_profile_hook = lambda: mod._hook
    mod.set_axon_ntff_profile_hook = lambda h: setattr(mod, "_hook", h)
    sys.modules["antenv.axon_hooks"] = mod


def kernel(cue, weights, noise, steps, cue_duration, trace=False):
    T = int(steps)
    cd = int(cue_duration)
    in_maps = prep_inputs(cue, weights, noise, T, cd)
    if T not in _NC_CACHE:
        nc_new = build_nc(T)
        nc_new.finalize()
        _NC_CACHE[T] = nc_new
    nc = _NC_CACHE[T]
    if trace:
        _ensure_ntff_hook()
    res = run_bass_kernel_spmd(nc, in_maps, list(range(NCORES)), trace=trace)
    out = assemble_output(res.results, T)
    kernel.last_result = res
    return out
